# revision 2
# baseline (speedup 1.0000x reference)
"""LinearAttention TRN2 kernel v2: data-parallel over batch on 8 cores.

Math (validated vs reference in fp64):
  Wq' = per-head Wq @ P (feature map folded into the Q projection); same for K.
  QkT = relu(Wq'^T q^T + bq)    [HF, tok]   (transposed activations)
  Ksum[hf,b] = sum_s relu(Wk'^T k^T + bk)   (Act accum per batch)
  U^T[c,b,h] = sum_d WvT[hd,c] Ksum[hd,b]   (V projection folded away)
  SrowT[v,h] = sum_c value[b,v,c] U^T[c,b,h]
  Z[tok] = per-head column sums of QkT;  zrec = 1/(Z + (F+1)eps)
  outT = QkT * Srow * zrec ; finT = Wo^T outT + bo

Precision scheme (rel-l2 ~0.0063 vs fp64 reference):
  K-proj:   single fp8e4 DoubleRow GEMM (quant noise averages out in the
            positive token-sum that produces Ksum).
  Q-proj,
  out-proj: 3-term compensated fp8e4 DoubleRow: X@W ~= X8@W8 + X8@Wr + Xr@W8
            where X8=e4m3(sx*X), Xr=e4m3(sx*X-X8); power-of-2 scales keep all
            operands inside e4m3's normal range and are folded into act/ts
            epilogues.
  V path:   bf16 (its error passes ~1:1 to the output).
"""
import numpy as np
import ml_dtypes

B, S, D, H = 64, 256, 2048, 8
DK = D // H
F = 256
EPS = 1e-8
NCORES = 8
BL = B // NCORES          # 8 batches per core
M = BL * S                # 2048 tokens per core
KT = D // 128             # 16 k-tiles
NSTRIP = 4                # strips of 512 tokens (2 batches)
SW = M // NSTRIP          # 512

SXQ, SWQ = 8.0, 16.0      # host pre-scales for fp8 operands
SXK, SWK = 4.0, 16.0
SO = 1.0 / 32             # outT quantization scale (on-device)
SWO = 256.0


def _build(rep=1):
    import concourse.bass as bass
    import concourse.mybir as mybir
    import concourse.tile as tile_mod
    from concourse.vector_clock import ScopedClock

    # ---- workaround: this walrus build allows ONE sync wait per instruction.
    if not getattr(tile_mod, "_onewait_patched", False):
        _orig_add = tile_mod.TileContext._add_instruction

        def _patched_add(self, inst):
            si = inst.sync_info
            if si is not None and si.on_wait is not None and len(si.on_wait) > 1:
                waits = list(si.on_wait)
                for w in waits[:-1]:
                    nop = mybir.InstNoOp(name=self.nc.get_next_instruction_name())
                    nop.engine = inst.engine
                    nop.sync_info = mybir.SyncInfo(on_wait=[w], on_update=[])
                    _orig_add(self, nop)
                inst.sync_info = mybir.SyncInfo(
                    on_wait=[waits[-1]], on_update=list(si.on_update)
                )
            _orig_add(self, inst)

        def _patched_drain(self, tick_clock, wait_clock):
            gc = tick_clock.global_clock
            items = gc.items() if hasattr(gc, "items") else [(None, gc)]
            for scope, vc in items:
                for proc in range(len(vc)):
                    t = vc[proc]
                    if t > 0:
                        nop = self.nc.sync.nop()
                        req = ScopedClock()
                        req.require_at_least(scope, proc, t)
                        wait_clock.add_sem_waits(nop.ins, req)
            self.nc.sync.drain()
            self.nc.all_engine_barrier()
            popped = self.nc._tile_sem_poison_stack.pop()
            assert popped is self._sem_poison
            self.nc.clear_and_free_semaphores(list(self.sems.allocated().values()))
            self.nc.all_engine_barrier()

        tile_mod.TileContext._add_instruction = _patched_add
        tile_mod.TileContext._drain_and_barrier = _patched_drain
        tile_mod._onewait_patched = True

    f32 = mybir.dt.float32
    bf16 = mybir.dt.bfloat16
    fp8 = mybir.dt.float8e4
    Relu = mybir.ActivationFunctionType.Relu
    Copy = mybir.ActivationFunctionType.Copy
    Alu = mybir.AluOpType
    DR = mybir.MatmulPerfMode.DoubleRow

    nc = bass.Bass()
    xq8 = nc.declare_dram_parameter("xq8", [D, M], fp8, isOutput=False)
    xqr = nc.declare_dram_parameter("xqr", [D, M], fp8, isOutput=False)
    xk8 = nc.declare_dram_parameter("xk8", [D, M], fp8, isOutput=False)
    xv = nc.declare_dram_parameter("xv", [D, M], bf16, isOutput=False)
    wq8 = nc.declare_dram_parameter("wq8", [KT, 128, D], fp8, isOutput=False)
    wqr = nc.declare_dram_parameter("wqr", [KT, 128, D], fp8, isOutput=False)
    wk8 = nc.declare_dram_parameter("wk8", [KT, 128, D], fp8, isOutput=False)
    wvt = nc.declare_dram_parameter("wvt", [KT, 128, D], bf16, isOutput=False)
    wo8 = nc.declare_dram_parameter("wo8", [KT, 128, D], fp8, isOutput=False)
    wor = nc.declare_dram_parameter("wor", [KT, 128, D], fp8, isOutput=False)
    sosel = nc.declare_dram_parameter("sosel", [8, H * 128], bf16, isOutput=False)
    bqp = nc.declare_dram_parameter("bqp", [D], f32, isOutput=False)
    bkp = nc.declare_dram_parameter("bkp", [D], f32, isOutput=False)
    bob = nc.declare_dram_parameter("bob", [D], f32, isOutput=False)
    fin = nc.declare_dram_parameter("fin", [D, M], bf16, isOutput=True)

    def r128(t):
        return t.rearrange("(t p) m -> p t m", p=128)

    with tile_mod.TileContext(nc) as tc:
        with (
            nc.allow_low_precision(reason="fp8/bf16 pipeline by design"),
            tc.tile_pool(name="persist", bufs=1) as ppool,
            tc.tile_pool(name="wbig", bufs=1) as wpool,
            tc.tile_pool(name="wchunk", bufs=4) as wcpool,
            tc.tile_pool(name="wvpool", bufs=4) as wvpool,
            tc.tile_pool(name="xstrip", bufs=2) as xpool,
            tc.tile_pool(name="xkpool", bufs=4) as xkpool,
            tc.tile_pool(name="xrpool", bufs=1) as xrpool,
            tc.tile_pool(name="qkpool", bufs=1) as qkpool,
            tc.tile_pool(name="otpool", bufs=2) as otpool,
            tc.tile_pool(name="scratch", bufs=2) as spool,
            tc.tile_pool(name="zpool", bufs=2) as zpool,
            tc.tile_pool(name="psbig", bufs=4, space="PSUM") as psbig,
            tc.tile_pool(name="psbpool", bufs=2, space="PSUM") as psbpool,
            tc.tile_pool(name="pssmall", bufs=2, space="PSUM") as pssmall,
        ):
            # persistent constants / small state
            bq_sb = ppool.tile([128, KT], f32, tag="bq")
            bk_sb = ppool.tile([128, KT], f32, tag="bk")
            bo_sb = ppool.tile([128, KT], f32, tag="bo")
            nc.sync.dma_start(bq_sb[:], bqp.rearrange("(t p) -> p t", p=128))
            nc.sync.dma_start(bk_sb[:], bkp.rearrange("(t p) -> p t", p=128))
            nc.sync.dma_start(bo_sb[:], bob.rearrange("(t p) -> p t", p=128))
            # zind[:, 7-h:15-h] is a [128,8] matrix whose column h is ones
            zind = ppool.tile([128, 15], bf16, tag="zind")
            nc.vector.memset(zind[:], 0.0)
            nc.vector.memset(zind[:, 7:8], 1.0)
            # so_sel[:, h, :]: [16,128] selector, row h = SO (broadcast head h)
            so_sel = ppool.tile([8, H, 128], bf16, tag="sosel")
            nc.sync.dma_start(so_sel[:], sosel[:, :].rearrange("p (h c) -> p h c", h=H))

            ksum = ppool.tile([128, KT, BL], f32, tag="ksum")
            ksum_bf = ppool.tile([128, KT, BL], bf16, tag="ksumbf")
            ut_sb = ppool.tile([128, KT, 64], bf16, tag="ut")
            srow = ppool.tile([128, 2, BL, H], f32, tag="srow")

            # big weights (wq8/wqr resident for all 4 strips; wk/wo streamed)
            wqt = wpool.tile([128, KT, KT, 128], fp8, tag="wq")
            wqrt = wpool.tile([128, KT, KT, 128], fp8, tag="wqr")

            for r in range(rep):
                # ---------------- phase B: K-proj -> Ksum (fp8 DR) ---------
                # t-outer: wk streamed once; all 4 xk strips resident.
                xss = []
                wk_pre = []
                for n in range(NSTRIP):
                    xs = xkpool.tile([128, KT, SW], fp8, tag="xk")
                    nc.sync.dma_start(xs[:], r128(xk8)[:, :, n * SW:(n + 1) * SW])
                    xss.append(xs)
                    if n == 0:
                        for tt in range(2):
                            wc = wcpool.tile([128, KT, 128], fp8, tag="wc8")
                            nc.sync.dma_start(wc[:], wk8[tt])
                            wk_pre.append(wc)
                for t in range(KT):
                    if t < 2:
                        wk_c = wk_pre[t]
                    else:
                        wk_c = wcpool.tile([128, KT, 128], fp8, tag="wc8")
                        nc.sync.dma_start(wk_c[:], wk8[t])
                    # trickle wq/wqr chunks in behind wk (2 per t)
                    nc.sync.dma_start(wqt[:, t, :, :], wq8[t])
                    nc.sync.dma_start(wqrt[:, t, :, :], wqr[t])
                    for n in range(NSTRIP):
                        ps = psbig.tile([128, SW], f32, tag="big")
                        for j in range(8):
                            nc.tensor.matmul(ps[:], wk_c[:, 2 * j:2 * j + 2, :],
                                             xss[n][:, 2 * j:2 * j + 2, :],
                                             start=(j == 0), stop=(j == 7),
                                             perf_mode=DR)
                        scrap = spool.tile([128, 256, 2], bf16, tag="d1")
                        for half in range(2):
                            b = 2 * n + half
                            nc.scalar.activation(
                                scrap[:, :, half], ps[:, half * 256:(half + 1) * 256],
                                Relu, bias=bk_sb[:, t:t + 1], scale=1.0 / (SXK * SWK),
                                accum_out=ksum[:, t, b:b + 1])
                nc.vector.tensor_scalar(ksum_bf[:], ksum[:], S * EPS, None, Alu.add)

                def a_gemm(n, per_t=None):
                    xs8 = xpool.tile([128, KT, SW], fp8, tag="xs")
                    xsr = xrpool.tile([128, KT, SW], fp8, tag="xsr")
                    nc.sync.dma_start(xs8[:], r128(xq8)[:, :, n * SW:(n + 1) * SW])
                    nc.sync.dma_start(xsr[:], r128(xqr)[:, :, n * SW:(n + 1) * SW])
                    qk = qkpool.tile([128, KT, SW], bf16, tag="qk")
                    for t in range(KT):
                        if per_t is not None:
                            per_t(t)
                        ps = psbig.tile([128, SW], f32, tag="big")
                        for j in range(8):
                            nc.tensor.matmul(ps[:], wqt[:, t, 2 * j:2 * j + 2, :],
                                             xs8[:, 2 * j:2 * j + 2, :],
                                             start=(j == 0), stop=False, perf_mode=DR)
                        for j in range(8):
                            nc.tensor.matmul(ps[:], wqrt[:, t, 2 * j:2 * j + 2, :],
                                             xs8[:, 2 * j:2 * j + 2, :],
                                             start=False, stop=False, perf_mode=DR)
                        for j in range(8):
                            nc.tensor.matmul(ps[:], wqt[:, t, 2 * j:2 * j + 2, :],
                                             xsr[:, 2 * j:2 * j + 2, :],
                                             start=False, stop=(j == 7), perf_mode=DR)
                        nc.scalar.activation(qk[:, t, :], ps[:], Relu,
                                             bias=bq_sb[:, t:t + 1],
                                             scale=1.0 / (SXQ * SWQ))
                    return qk

                def a_scale_blocks(n, qk):
                    '''Returns (out8, outr, block_fn); block_fn(i) for i=0..8
                    emits the Z stage (i=0) or head i-1's scale work. Designed
                    to interleave into a following PE-heavy loop.'''
                    out8 = otpool.tile([128, KT, SW], fp8, tag="o8")
                    outr = otpool.tile([128, KT, SW], fp8, tag="or")
                    state = {}

                    def block(i):
                        if i == 0:
                            pszall = pssmall.tile([8, SW], f32, tag="small")
                            for t in range(KT):
                                h = t // 2
                                nc.tensor.matmul(pszall[:], zind[:, 7 - h:15 - h],
                                                 qk[:, t, :],
                                                 start=(t == 0), stop=(t == KT - 1))
                            zrall = zpool.tile([8, SW], bf16, tag="zr")
                            nc.vector.reciprocal(zrall[:], pszall[:])
                            state['zr'] = zrall
                            return
                        if i > H:
                            return
                        h = i - 1
                        psb = psbpool.tile([128, SW], f32, tag="psb")
                        nc.tensor.matmul(psb[:], so_sel[:, h, :], state['zr'][:],
                                         start=True, stop=True)
                        for fh in range(2):
                            t = 2 * h + fh
                            d1 = spool.tile([128, SW], f32, tag="d1")
                            for half in range(2):
                                b = 2 * n + half
                                sl = slice(half * 256, (half + 1) * 256)
                                nc.vector.scalar_tensor_tensor(
                                    d1[:, sl], qk[:, t, sl],
                                    srow[:, fh, b, h:h + 1], psb[:, sl],
                                    Alu.mult, Alu.mult)
                            nc.scalar.activation(out8[:, t, :], d1[:], Copy)
                            nc.vector.tensor_tensor(outr[:, t, :], d1[:], out8[:, t, :],
                                                    Alu.subtract)

                    return out8, outr, block

                def d_prefetch():
                    wo_c = wcpool.tile([128, KT, 128], fp8, tag="wc8")
                    wor_c = wcpool.tile([128, KT, 128], fp8, tag="wc8")
                    nc.sync.dma_start(wo_c[:], wo8[0])
                    nc.sync.dma_start(wor_c[:], wor[0])
                    return wo_c, wor_c

                def d_proj_pair(strips, prefetched=None, per_m=None):
                    for m in range(KT):
                        if per_m is not None:
                            per_m(m)
                        if m == 0 and prefetched is not None:
                            wo_c, wor_c = prefetched
                        else:
                            wo_c = wcpool.tile([128, KT, 128], fp8, tag="wc8")
                            wor_c = wcpool.tile([128, KT, 128], fp8, tag="wc8")
                            nc.sync.dma_start(wo_c[:], wo8[m])
                            nc.sync.dma_start(wor_c[:], wor[m])
                        for n, out8, outr in strips:
                            ps = psbig.tile([128, SW], f32, tag="big")
                            for j in range(8):
                                nc.tensor.matmul(ps[:], wo_c[:, 2 * j:2 * j + 2, :],
                                                 out8[:, 2 * j:2 * j + 2, :],
                                                 start=(j == 0), stop=False, perf_mode=DR)
                            for j in range(8):
                                nc.tensor.matmul(ps[:], wor_c[:, 2 * j:2 * j + 2, :],
                                                 out8[:, 2 * j:2 * j + 2, :],
                                                 start=False, stop=False, perf_mode=DR)
                            for j in range(8):
                                nc.tensor.matmul(ps[:], wo_c[:, 2 * j:2 * j + 2, :],
                                                 outr[:, 2 * j:2 * j + 2, :],
                                                 start=False, stop=(j == 7), perf_mode=DR)
                            fo = zpool.tile([128, SW], bf16, tag="fo")
                            nc.vector.tensor_scalar(fo[:], ps[:], 1.0 / (SO * SWO),
                                                    bo_sb[:, m:m + 1], Alu.mult, Alu.add)
                            nc.sync.dma_start(
                                fin[m * 128:(m + 1) * 128, n * SW:(n + 1) * SW], fo[:])

                # ---- U^T chunk worker (wvt streamed; interleaved into A0) ----
                def ut_chunk(ct2):
                    ct, piece = ct2 // 2, ct2 % 2
                    if ct2 % 2 == 0:
                        wv_c = wvpool.tile([128, KT, 64], bf16, tag="wvc")
                    else:
                        wv_c = wcpool.tile([128, KT, 64], bf16, tag="wc8")
                    nc.sync.dma_start(
                        wv_c[:], wvt[ct].rearrange("p (t c) -> p t c", t=KT)[:, :, piece * 64:(piece + 1) * 64])
                    psu = pssmall.tile([128, 64], f32, tag="small")
                    for h in range(H):
                        for j in range(2):
                            t = 2 * h + j
                            nc.tensor.matmul(psu[:64, h * 8:(h + 1) * 8],
                                             wv_c[:, t, :], ksum_bf[:, t, :],
                                             start=(j == 0), stop=(j == 1))
                    nc.vector.tensor_copy(ut_sb[piece * 64:(piece + 1) * 64, ct, :], psu[:64, :])

                # A strip 0 GEMM with U^T interleaved (2 chunks per t)
                qk0 = a_gemm(0, per_t=lambda t: [ut_chunk(2 * t), ut_chunk(2 * t + 1)])
                def srow_calc(b0, b1):
                    for b in range(b0, b1):
                        xsv = xpool.tile([128, KT, S], bf16, tag="xs")
                        nc.sync.dma_start(xsv[:], r128(xv)[:, :, b * S:(b + 1) * S])
                        for vch in range(2):
                            pss = pssmall.tile([128, 8], f32, tag="small")
                            for ct in range(KT):
                                nc.tensor.matmul(pss[:], xsv[:, ct, vch * 128:(vch + 1) * 128],
                                                 ut_sb[:, ct, b::8],
                                                 start=(ct == 0), stop=(ct == KT - 1))
                            nc.vector.tensor_copy(srow[:, vch, b, :], pss[:])

                # software-pipelined A/D: each strip's scale work interleaves
                # into the NEXT PE-heavy block so PE never waits on DVE.
                srow_calc(0, 2)
                o80, or0, blk0 = a_scale_blocks(0, qk0)

                def per_t1(t):
                    blk0(t)
                    if t < 6:
                        srow_calc(2 + t, 3 + t)
                qk1 = a_gemm(1, per_t=per_t1)
                o81, or1, blk1 = a_scale_blocks(1, qk1)
                qk2 = a_gemm(2, per_t=blk1)
                pf = d_prefetch()
                o82, or2, blk2 = a_scale_blocks(2, qk2)
                d_proj_pair([(0, o80, or0), (1, o81, or1)], prefetched=pf)
                qk3 = a_gemm(3, per_t=blk2)
                pf2 = d_prefetch()
                o83, or3, blk3 = a_scale_blocks(3, qk3)
                d_proj_pair([(2, o82, or2)], prefetched=pf2, per_m=blk3)
                d_proj_pair([(3, o83, or3)])
    return nc


_NC = None


def _prep_host(query, key, value, Wq, bq, Wk, bk, Wv, bv, Wo, bo, random_proj):
    bf = ml_dtypes.bfloat16
    f8 = ml_dtypes.float8_e4m3

    def fold(Wx):
        return np.einsum('dhk,kf->dhf', Wx.reshape(D, H, DK).astype(np.float32),
                         random_proj.astype(np.float32)).reshape(D, D)

    def split8(x, s):
        xs = np.asarray(x, np.float32) * s
        hi = xs.astype(f8)
        lo = (xs - hi.astype(np.float32)).astype(f8)
        return hi, lo

    Wqp = fold(Wq)
    Wkp = fold(Wk)
    bqp = ((bq.reshape(H, DK) @ random_proj).reshape(D)).astype(np.float32)
    bkp = ((bk.reshape(H, DK) @ random_proj).reshape(D)).astype(np.float32)

    def chunked(Wmat):
        # [t, p, k, c] layout: chunk t reads contiguous (k, c) rows per p
        return np.ascontiguousarray(
            Wmat.reshape(KT, 128, KT, 128).transpose(2, 1, 0, 3).reshape(KT, 128, D))

    wq8f, wqrf = split8(Wqp, SWQ)
    wq8, wqr = chunked(wq8f), chunked(wqrf)
    wk8 = chunked((Wkp * SWK).astype(f8))
    wo8f, worf = split8(Wo, SWO)
    wo8, wor = chunked(wo8f), chunked(worf)
    wvt_b = chunked(np.ascontiguousarray(Wv.T).astype(bf))

    sosel = np.zeros((8, H, 128), np.float32)
    for h in range(H):
        sosel[h, h, :] = SO
    sosel = sosel.reshape(8, H * 128).astype(bf)

    weights = {
        "sosel": sosel,
        "wq8": wq8, "wqr": wqr,
        "wk8": np.ascontiguousarray(wk8),
        "wvt": wvt_b,
        "wo8": np.ascontiguousarray(wo8), "wor": np.ascontiguousarray(wor),
        "bqp": bqp, "bkp": bkp, "bob": bo.astype(np.float32),
    }
    in_maps = []
    for c in range(NCORES):
        sl = slice(c * BL, (c + 1) * BL)
        qT = np.ascontiguousarray(query[sl].reshape(M, D).T)
        kT = np.ascontiguousarray(key[sl].reshape(M, D).T)
        vT = np.ascontiguousarray(value[sl].reshape(M, D).T)
        q8c, qrc = split8(qT, SXQ)
        in_maps.append({
            "xq8": q8c, "xqr": qrc,
            "xk8": (kT * SXK).astype(f8),
            "xv": vT.astype(bf),
            **weights,
        })
    return in_maps


def kernel(query, key, value, Wq, bq, Wk, bk, Wv, bv, Wo, bo, random_proj):
    global _NC
    from concourse.bass_utils import run_bass_kernel_spmd

    in_maps = _prep_host(query, key, value, Wq, bq, Wk, bk, Wv, bv, Wo, bo,
                         random_proj)
    if _NC is None:
        _NC = _build(rep=1)
    res = run_bass_kernel_spmd(_NC, in_maps, list(range(NCORES)))
    out = np.empty((B, S, D), dtype=np.float32)
    for c in range(NCORES):
        finT = res.results[c]["fin"]                      # [D, M] bf16
        out[c * BL:(c + 1) * BL] = finT.astype(np.float32).T.reshape(BL, S, D)
    kernel._last_in_maps = in_maps
    return out
